# revision 28
# baseline (speedup 1.0000x reference)
"""AdditiveAttention TRN2 kernel (8 NeuronCores, data-parallel over batch).

Reference computation (B=32, S=D=1024):
    q = x @ Wq^T + bq;  k = x @ Wk^T + bk
    scores = tanh(q @ k^T);  s = scores @ v
    w = softmax(s);  out = w @ x          -> [B, D]

Algebraic restructure: q @ k^T = x M x^T + a 1^T + 1 b^T + c, with
    M = Wq^T Wk,  a = x (Wq^T bk),  b = x (Wk^T bq),  c = bq.bk
so only TWO big matmuls per batch are needed on-device:
    y^T = M^T x^T   (fold b via per-partition bias)
    G^T = x y'^T    (fold a via rank-1 accumulate matmul)
then s = v^T tanh(G^T), w = softmax(s), out = sum_s w[s] x[:,s].
All big matmuls run in float32r (TF32-like: 11 mantissa bits, 4x faster
than fp32 on the PE). Inputs are pre-rounded host-side (RNE dropping 12
mantissa bits — bit-exact match with the hardware's fp32->fp32r cast).
"""
import numpy as np

import concourse.bass as bass
import concourse.bacc as bacc
import concourse.mybir as mybir
import concourse.tile as tile
from concourse.bass_utils import run_bass_kernel_spmd

B, S, D = 32, 1024, 1024
NCORES = 8
BL = B // NCORES          # batches per core
PT = 128                  # partition tile
ND = D // PT              # feature tiles
SC = 512                  # s-chunk (PSUM bank limit for 4-byte dtypes)
NSC = S // SC

f32 = mybir.dt.float32
f32r = mybir.dt.float32r
AF = mybir.ActivationFunctionType
ALU = mybir.AluOpType
AX = mybir.AxisListType
bf16 = mybir.dt.bfloat16


def _rne12(a: np.ndarray) -> np.ndarray:
    """Round fp32 to f32r (RNE, drop 12 mantissa bits) — matches TRN2's cast."""
    bits = np.ascontiguousarray(a, dtype=np.float32).view(np.uint32)
    r = bits + np.uint32(1 << 11) - np.uint32(1) + ((bits >> np.uint32(12)) & np.uint32(1))
    return (r & ~np.uint32((1 << 12) - 1)).view(np.float32)


def _build(with_u2: bool, with_a: bool, c_bias: float = 0.0):
    nc = bacc.Bacc("TRN2", target_bir_lowering=False, debug=False)
    xt_d = nc.declare_dram_parameter("xt", [BL, D, S], f32r, isOutput=False)
    # m in [dp, dk, 128, 128] blocks: blk[dp, dk] = M[dk*128:.., dp*128:..]
    m_d = nc.declare_dram_parameter("m", [ND, ND, PT, PT], f32r, isOutput=False)
    vr_d = nc.declare_dram_parameter("vr", [PT, ND], f32r, isOutput=False)
    if with_u2:
        u2_d = nc.declare_dram_parameter("u2r", [PT, ND], f32, isOutput=False)
    if with_a:
        u1_d = nc.declare_dram_parameter("u1r", [PT, ND], f32r, isOutput=False)
    out_d = nc.declare_dram_parameter("out", [BL, D], f32, isOutput=True)
    zn_d = nc.declare_dram_parameter("zn", [BL, 1], f32, isOutput=True)

    with tile.TileContext(nc) as tc:
        with (
            tc.tile_pool(name="consts", bufs=1) as consts,
            tc.tile_pool(name="xt", bufs=2 * ND) as xt_pool,
            tc.tile_pool(name="y", bufs=ND) as y_pool,
            tc.tile_pool(name="tt", bufs=5) as t_pool,
            tc.tile_pool(name="rows", bufs=2) as row_pool,
            tc.tile_pool(name="small", bufs=4) as small_pool,
            tc.tile_pool(name="scr", bufs=2) as scr_pool,
            tc.tile_pool(name="oc", bufs=2) as oc_pool,
            tc.tile_pool(name="psy", bufs=(1 if with_a else 2), space="PSUM") as psy_pool,
            tc.tile_pool(name="psg", bufs=2, space="PSUM") as psg_pool,
            tc.tile_pool(name="psv", bufs=1, space="PSUM") as psv_pool,
            tc.tile_pool(name="psw", bufs=1, space="PSUM") as psw_pool,
        ):
            # PE warmup source (ready ~immediately): flips HAM to 2.4 GHz
            # while the initial DMAs are still in flight.
            ones_f32 = consts.tile([1, PT], f32, tag="ones32")
            nc.vector.memset(ones_f32[:], 1.0)
            ones_sb = consts.tile([1, PT], f32r, tag="ones")
            nc.vector.tensor_copy(ones_sb[:], ones_f32[:])
            wsrc_f32 = consts.tile([1, SC], f32, tag="wsrc32")
            nc.vector.memset(wsrc_f32[:], 0.0)
            wsrc = consts.tile([1, SC], f32r, tag="wsrc")
            nc.vector.tensor_copy(wsrc[:], wsrc_f32[:])
            onescol_f32 = consts.tile([PT, 1], f32, tag="onescol32")
            nc.vector.memset(onescol_f32[:], 1.0)
            onescol = consts.tile([PT, 1], f32r, tag="onescol")
            nc.vector.tensor_copy(onescol[:], onescol_f32[:])
            pwarm = psy_pool.tile([1, SC], f32, tag="py", name="pwarm")
            for _ in range(12):
                nc.tensor.matmul(pwarm[:], wsrc[:, 0:1], wsrc[:],
                                 start=True, stop=True)

            # resident constants; m arrives in [dp, dk] 64 KiB blocks, dp-major
            # and interleaved with batch-0 x so the PE can start early.
            m_sb = [consts.tile([PT, D], f32r, tag=f"m{dk}", name=f"m{dk}")
                    for dk in range(ND)]
            xt0_sb = [xt_pool.tile([PT, S], f32r, tag="xt", name=f"xt0_{dk}")
                      for dk in range(ND)]
            def xt0_half(dk, sc):
                nc.sync.dma_start(
                    xt0_sb[dk][:, sc * SC:(sc + 1) * SC],
                    xt_d.ap()[0, dk * PT:(dk + 1) * PT, sc * SC:(sc + 1) * SC])
            for dp in range(ND):
                for dk in range(ND):
                    nc.sync.dma_start(
                        m_sb[dk][:, dp * PT:(dp + 1) * PT], m_d.ap()[dp, dk])
                    if dp == 0:
                        # all batch-0 sc0 halves first: phase A runs sc0
                        # groups while the rest streams in
                        xt0_half(dk, 0)
                if dp >= 1:
                    xt0_half(dp - 1, 1)
            xt0_half(ND - 1, 1)
            vr_sb = consts.tile([PT, ND], f32r, tag="vr")
            nc.sync.dma_start(vr_sb[:], vr_d.ap()[:])
            if with_u2:
                u2_sb = consts.tile([PT, ND], f32, tag="u2")
                nc.sync.dma_start(u2_sb[:], u2_d.ap()[:])
            if with_a:
                u1_sb = consts.tile([PT, ND], f32r, tag="u1")
                nc.sync.dma_start(u1_sb[:], u1_d.ap()[:])

            for b in range(BL):
                if b == 0:
                    xt_sb = xt0_sb
                else:
                    xt_sb = []
                    for dk in range(ND):
                        t = xt_pool.tile([PT, S], f32r, tag="xt", name=f"xt{b}_{dk}")
                        nc.sync.dma_start(t[:], xt_d.ap()[b, dk * PT:(dk + 1) * PT, :])
                        xt_sb.append(t)

                # ---- Phase A: y'^T[d', s] = sum_d M[d, d'] X[d, s] (+ u2[d'])
                y_sb = [y_pool.tile([PT, S], f32r, tag="y", name=f"y{b}_{i}") for i in range(ND)]
                if b == 0:
                    groups = [(dp, sc) for sc in range(NSC) for dp in range(ND)]
                else:
                    groups = [(dp, sc) for dp in range(ND) for sc in range(NSC)]
                for dp, sc in groups:
                    if True:
                        py = psy_pool.tile([PT, SC], f32, tag="py", name=f"py{b}_{dp}_{sc}")
                        for dk in range(ND):
                            nc.tensor.matmul(
                                py[:],
                                m_sb[dk][:, dp * PT:(dp + 1) * PT],
                                xt_sb[dk][:, sc * SC:(sc + 1) * SC],
                                start=(dk == 0), stop=(dk == ND - 1),
                            )
                        dst = y_sb[dp][:, sc * SC:(sc + 1) * SC]
                        if with_u2:
                            nc.scalar.activation(dst, py[:], AF.Identity,
                                                 bias=u2_sb[:, dp:dp + 1])
                        else:
                            nc.scalar.activation(dst, py[:], AF.Copy)

                # ---- optional a-row: a[s] = sum_d X[d,s] u1[d] + c
                if with_a:
                    arow = row_pool.tile([1, S], f32r, tag="arow", name=f"arow{b}")
                    for sc in range(NSC):
                        pa = psy_pool.tile([1, SC], f32, tag="pa", name=f"pa{b}_{sc}")
                        for dk in range(ND):
                            nc.tensor.matmul(
                                pa[:],
                                u1_sb[:, dk:dk + 1],
                                xt_sb[dk][:, sc * SC:(sc + 1) * SC],
                                start=(dk == 0), stop=(dk == ND - 1),
                            )
                        nc.scalar.activation(
                            arow[:, sc * SC:(sc + 1) * SC], pa[:], AF.Copy,
                            bias=c_bias)

                # ---- Phase B: G^T[t,s] = sum_d' X[d',t] y'[d',s]; tanh;
                # v-weighted partial sums accumulate on the DVE (keeps PE free)
                acc = [scr_pool.tile([PT, SC], f32, tag=f"acc{sc}",
                                     name=f"acc{b}_{sc}") for sc in range(NSC)]
                accr = [t_pool.tile([PT, SC], f32r, tag="tT",
                                    name=f"accr{b}_{sc}") for sc in range(NSC)]
                for ttile in range(ND):
                    for sc in range(NSC):
                        pg = psg_pool.tile([PT, SC], f32, tag="pg", name=f"pg{b}_{ttile}_{sc}")
                        for dk in range(ND):
                            nc.tensor.matmul(
                                pg[:],
                                xt_sb[dk][:, ttile * PT:(ttile + 1) * PT],
                                y_sb[dk][:, sc * SC:(sc + 1) * SC],
                                start=(dk == 0),
                                stop=(dk == ND - 1) and not with_a,
                            )
                        if with_a:
                            nc.tensor.matmul(
                                pg[:], ones_sb[:],
                                arow[:, sc * SC:(sc + 1) * SC],
                                start=False, stop=True,
                            )
                        tT = t_pool.tile([PT, SC], f32r, tag="tT", name=f"tT{b}_{ttile}_{sc}")
                        nc.scalar.activation(tT[:], pg[:], AF.Tanh)
                        vcol = vr_sb[:, ttile:ttile + 1].bitcast(f32)
                        if ttile == 0:
                            nc.vector.tensor_scalar_mul(
                                acc[sc][:], tT[:].bitcast(f32), vcol)
                        elif ttile < ND - 1:
                            nc.vector.scalar_tensor_tensor(
                                acc[sc][:], tT[:].bitcast(f32), vcol,
                                acc[sc][:], op0=ALU.mult, op1=ALU.add)
                        else:
                            # final accumulate writes the f32r matmul operand
                            nc.vector.scalar_tensor_tensor(
                                accr[sc][:], tT[:].bitcast(f32), vcol,
                                acc[sc][:], op0=ALU.mult, op1=ALU.add)
                # cross-partition sum of acc via ones-column matmul
                sv = psv_pool.tile([1, S], f32, tag="sv", name=f"sv{b}")
                for sc in range(NSC):
                    nc.tensor.matmul(sv[:, sc * SC:(sc + 1) * SC],
                                     onescol[:], accr[sc][:],
                                     start=True, stop=True)

                # ---- Phase C: softmax over sv row; out = sum_s w[s] X[:, s]
                negm = small_pool.tile([1, 1], f32, tag="negm", name=f"negm{b}")
                nc.vector.reduce_max(negm[:], sv[:], axis=AX.X, negate=True)
                erow = row_pool.tile([1, S], f32r, tag="erow", name=f"erow{b}")
                zsum = small_pool.tile([1, 1], f32, tag="zsum", name=f"zsum{b}")
                nc.scalar.activation(erow[:], sv[:], AF.Exp,
                                     bias=negm[:], accum_out=zsum[:])
                # normalization (the 1/Z divide) happens on the host:
                # broadcast unnormalized exp weights, emit Z separately
                nc.sync.dma_start(zn_d.ap()[b:b + 1, :], zsum[:])
                pw = psw_pool.tile([PT, S], f32, tag="pw", name=f"pw{b}")
                for sc in range(NSC):
                    nc.tensor.matmul(
                        pw[:, sc * SC:(sc + 1) * SC],
                        ones_sb[:],
                        erow[:, sc * SC:(sc + 1) * SC],
                        start=True, stop=True,
                    )
                # fused multiply+free-dim-sum on the DVE, reading pw PSUM
                oc = oc_pool.tile([PT, ND], f32, tag="oc", name=f"oc{b}")
                for dk in range(ND):
                    scr = scr_pool.tile([PT, S], f32, tag="scr", name=f"scr{b}_{dk}")
                    nc.vector.scalar_tensor_tensor(
                        scr[:], xt_sb[dk][:].bitcast(f32), 1.0, pw[:],
                        op0=ALU.mult, op1=ALU.mult,
                        accum_out=oc[:, dk:dk + 1])
                nc.sync.dma_start(
                    out_d.ap()[b].rearrange("(i p) -> p i", p=PT), oc[:])

    nc.compile()
    return nc


_CACHE: dict = {}


def _get_nc(with_u2: bool, with_a: bool, c_bias: float):
    key = (with_u2, with_a, c_bias if with_a else 0.0)
    if key not in _CACHE:
        _CACHE[key] = _build(with_u2, with_a, c_bias)
    return _CACHE[key]


def kernel(x, Wq, bq, Wk, bk, v):
    x = np.asarray(x, dtype=np.float32)
    Wq = np.asarray(Wq, dtype=np.float32)
    bq = np.asarray(bq, dtype=np.float32)
    Wk = np.asarray(Wk, dtype=np.float32)
    bk = np.asarray(bk, dtype=np.float32)
    v = np.asarray(v, dtype=np.float32)

    # host-side algebra (small, fp64 for accuracy)
    M = (Wq.astype(np.float64).T @ Wk.astype(np.float64)).astype(np.float32)
    u2 = (Wk.astype(np.float64).T @ bq.astype(np.float64)).astype(np.float32)
    u1 = (Wq.astype(np.float64).T @ bk.astype(np.float64)).astype(np.float32)
    c = float(bq.astype(np.float64) @ bk.astype(np.float64))

    with_u2 = bool(np.any(u2))
    with_a = bool(np.any(u1)) or c != 0.0

    # [dp, dk, 128, 128] blocks: blk[dp, dk] = M[dk*128:.., dp*128:..]
    m_blocks = np.ascontiguousarray(
        M.reshape(ND, PT, ND, PT).transpose(2, 0, 1, 3))
    m_r = _rne12(m_blocks)
    vr = _rne12(np.ascontiguousarray(v.reshape(ND, PT).T))
    u2r = np.ascontiguousarray(u2.reshape(ND, PT).T)
    u1r = _rne12(np.ascontiguousarray(u1.reshape(ND, PT).T))

    nc = _get_nc(with_u2, with_a, c)

    in_maps = []
    for core in range(NCORES):
        xs = x[core * BL:(core + 1) * BL]              # [BL, S, D]
        xts = _rne12(np.ascontiguousarray(xs.transpose(0, 2, 1)))  # [BL, D, S]
        im = {"xt": xts, "m": m_r, "vr": vr}
        if with_u2:
            im["u2r"] = u2r
        if with_a:
            im["u1r"] = u1r
        in_maps.append(im)

    global _LAST_IN_MAPS
    _LAST_IN_MAPS = in_maps
    last_exc = None
    for attempt in range(3):
        try:
            res = run_bass_kernel_spmd(nc, in_maps,
                                       core_ids=list(range(NCORES)),
                                       trace=False)
            break
        except Exception as e:  # transient device errors: back off and retry
            last_exc = e
            import time as _time
            _time.sleep(5 * (attempt + 1))
    else:
        raise last_exc
    out = np.concatenate([res.results[i]["out"] for i in range(NCORES)], axis=0)
    zn = np.concatenate([res.results[i]["zn"] for i in range(NCORES)], axis=0)
    out = out / zn
    return out.astype(np.float32)


# revision 29
# speedup vs baseline: 1.0644x; 1.0644x over previous
"""AdditiveAttention TRN2 kernel (8 NeuronCores, data-parallel over batch).

Reference computation (B=32, S=D=1024):
    q = x @ Wq^T + bq;  k = x @ Wk^T + bk
    scores = tanh(q @ k^T);  s = scores @ v
    w = softmax(s);  out = w @ x          -> [B, D]

Algebraic restructure: q @ k^T = x M x^T + a 1^T + 1 b^T + c, with
    M = Wq^T Wk,  a = x (Wq^T bk),  b = x (Wk^T bq),  c = bq.bk
so only TWO big matmuls per batch are needed on-device:
    y^T = M^T x^T   (fold b via per-partition bias)
    G^T = x y'^T    (fold a via rank-1 accumulate matmul)
then s = v^T tanh(G^T), w = softmax(s), out = sum_s w[s] x[:,s].
All big matmuls run in float32r (TF32-like: 11 mantissa bits, 4x faster
than fp32 on the PE). Inputs are pre-rounded host-side (RNE dropping 12
mantissa bits — bit-exact match with the hardware's fp32->fp32r cast).
"""
import numpy as np

import concourse.bass as bass
import concourse.bacc as bacc
import concourse.mybir as mybir
import concourse.tile as tile
from concourse.bass_utils import run_bass_kernel_spmd

B, S, D = 32, 1024, 1024
NCORES = 8
BL = B // NCORES          # batches per core
PT = 128                  # partition tile
ND = D // PT              # feature tiles
SC = 512                  # s-chunk (PSUM bank limit for 4-byte dtypes)
NSC = S // SC

f32 = mybir.dt.float32
f32r = mybir.dt.float32r
AF = mybir.ActivationFunctionType
ALU = mybir.AluOpType
AX = mybir.AxisListType
bf16 = mybir.dt.bfloat16


def _rne12(a: np.ndarray) -> np.ndarray:
    """Round fp32 to f32r (RNE, drop 12 mantissa bits) — matches TRN2's cast."""
    bits = np.ascontiguousarray(a, dtype=np.float32).view(np.uint32)
    r = bits + np.uint32(1 << 11) - np.uint32(1) + ((bits >> np.uint32(12)) & np.uint32(1))
    return (r & ~np.uint32((1 << 12) - 1)).view(np.float32)


def _build(with_u2: bool, with_a: bool, c_bias: float = 0.0):
    nc = bacc.Bacc("TRN2", target_bir_lowering=False, debug=False)
    xt_d = nc.declare_dram_parameter("xt", [BL, D, S], f32r, isOutput=False)
    # m in [dp, dk, 128, 128] blocks: blk[dp, dk] = M[dk*128:.., dp*128:..]
    m_d = nc.declare_dram_parameter("m", [ND, ND, PT, PT], f32r, isOutput=False)
    vr_d = nc.declare_dram_parameter("vr", [PT, ND], f32r, isOutput=False)
    if with_u2:
        u2_d = nc.declare_dram_parameter("u2r", [PT, ND], f32, isOutput=False)
    if with_a:
        u1_d = nc.declare_dram_parameter("u1r", [PT, ND], f32r, isOutput=False)
    out_d = nc.declare_dram_parameter("out", [BL, D], f32, isOutput=True)
    zn_d = nc.declare_dram_parameter("zn", [BL, 1], f32, isOutput=True)

    with tile.TileContext(nc) as tc:
        with (
            tc.tile_pool(name="consts", bufs=1) as consts,
            tc.tile_pool(name="xt", bufs=2 * ND) as xt_pool,
            tc.tile_pool(name="y", bufs=ND) as y_pool,
            tc.tile_pool(name="tt", bufs=5) as t_pool,
            tc.tile_pool(name="rows", bufs=2) as row_pool,
            tc.tile_pool(name="small", bufs=4) as small_pool,
            tc.tile_pool(name="scr", bufs=2) as scr_pool,
            tc.tile_pool(name="oc", bufs=2) as oc_pool,
            tc.tile_pool(name="psy", bufs=(1 if with_a else 2), space="PSUM") as psy_pool,
            tc.tile_pool(name="psg", bufs=2, space="PSUM") as psg_pool,
            tc.tile_pool(name="psv", bufs=1, space="PSUM") as psv_pool,
            tc.tile_pool(name="psw", bufs=1, space="PSUM") as psw_pool,
        ):
            # PE warmup source (ready ~immediately): flips HAM to 2.4 GHz
            # while the initial DMAs are still in flight.
            ones_f32 = consts.tile([1, PT], f32, tag="ones32")
            nc.vector.memset(ones_f32[:], 1.0)
            ones_sb = consts.tile([1, PT], f32r, tag="ones")
            nc.vector.tensor_copy(ones_sb[:], ones_f32[:])
            wsrc_f32 = consts.tile([1, SC], f32, tag="wsrc32")
            nc.vector.memset(wsrc_f32[:], 0.0)
            wsrc = consts.tile([1, SC], f32r, tag="wsrc")
            nc.vector.tensor_copy(wsrc[:], wsrc_f32[:])
            onescol_f32 = consts.tile([PT, 1], f32, tag="onescol32")
            nc.vector.memset(onescol_f32[:], 1.0)
            onescol = consts.tile([PT, 1], f32r, tag="onescol")
            nc.vector.tensor_copy(onescol[:], onescol_f32[:])
            pwarm = psy_pool.tile([1, SC], f32, tag="py", name="pwarm")
            for _ in range(12):
                nc.tensor.matmul(pwarm[:], wsrc[:, 0:1], wsrc[:],
                                 start=True, stop=True)

            # resident constants; m arrives in [dp, dk] 64 KiB blocks, dp-major
            # and interleaved with batch-0 x so the PE can start early.
            m_sb = [consts.tile([PT, D], f32r, tag=f"m{dk}", name=f"m{dk}")
                    for dk in range(ND)]
            xt0_sb = [xt_pool.tile([PT, S], f32r, tag="xt", name=f"xt0_{dk}")
                      for dk in range(ND)]
            for dp in range(ND):
                for dk in range(ND):
                    nc.sync.dma_start(
                        m_sb[dk][:, dp * PT:(dp + 1) * PT], m_d.ap()[dp, dk])
                    if dp < 2:
                        # batch-0 x half for s-chunk dp, paired with its m blk
                        nc.sync.dma_start(
                            xt0_sb[dk][:, dp * SC:(dp + 1) * SC],
                            xt_d.ap()[0, dk * PT:(dk + 1) * PT,
                                      dp * SC:(dp + 1) * SC])
            vr_sb = consts.tile([PT, ND], f32r, tag="vr")
            nc.sync.dma_start(vr_sb[:], vr_d.ap()[:])
            if with_u2:
                u2_sb = consts.tile([PT, ND], f32, tag="u2")
                nc.sync.dma_start(u2_sb[:], u2_d.ap()[:])
            if with_a:
                u1_sb = consts.tile([PT, ND], f32r, tag="u1")
                nc.sync.dma_start(u1_sb[:], u1_d.ap()[:])

            for b in range(BL):
                if b == 0:
                    xt_sb = xt0_sb
                else:
                    xt_sb = []
                    for dk in range(ND):
                        t = xt_pool.tile([PT, S], f32r, tag="xt", name=f"xt{b}_{dk}")
                        nc.sync.dma_start(t[:], xt_d.ap()[b, dk * PT:(dk + 1) * PT, :])
                        xt_sb.append(t)

                # ---- Phase A: y'^T[d', s] = sum_d M[d, d'] X[d, s] (+ u2[d'])
                y_sb = [y_pool.tile([PT, S], f32r, tag="y", name=f"y{b}_{i}") for i in range(ND)]
                for dp in range(ND):
                    for sc in range(NSC):
                        py = psy_pool.tile([PT, SC], f32, tag="py", name=f"py{b}_{dp}_{sc}")
                        for dk in range(ND):
                            nc.tensor.matmul(
                                py[:],
                                m_sb[dk][:, dp * PT:(dp + 1) * PT],
                                xt_sb[dk][:, sc * SC:(sc + 1) * SC],
                                start=(dk == 0), stop=(dk == ND - 1),
                            )
                        dst = y_sb[dp][:, sc * SC:(sc + 1) * SC]
                        if with_u2:
                            nc.scalar.activation(dst, py[:], AF.Identity,
                                                 bias=u2_sb[:, dp:dp + 1])
                        else:
                            nc.scalar.activation(dst, py[:], AF.Copy)

                # ---- optional a-row: a[s] = sum_d X[d,s] u1[d] + c
                if with_a:
                    arow = row_pool.tile([1, S], f32r, tag="arow", name=f"arow{b}")
                    for sc in range(NSC):
                        pa = psy_pool.tile([1, SC], f32, tag="pa", name=f"pa{b}_{sc}")
                        for dk in range(ND):
                            nc.tensor.matmul(
                                pa[:],
                                u1_sb[:, dk:dk + 1],
                                xt_sb[dk][:, sc * SC:(sc + 1) * SC],
                                start=(dk == 0), stop=(dk == ND - 1),
                            )
                        nc.scalar.activation(
                            arow[:, sc * SC:(sc + 1) * SC], pa[:], AF.Copy,
                            bias=c_bias)

                # ---- Phase B: G^T[t,s] = sum_d' X[d',t] y'[d',s]; tanh;
                # v-weighted partial sums accumulate on the DVE (keeps PE free)
                acc = [scr_pool.tile([PT, SC], f32, tag=f"acc{sc}",
                                     name=f"acc{b}_{sc}") for sc in range(NSC)]
                accr = [t_pool.tile([PT, SC], f32r, tag="tT",
                                    name=f"accr{b}_{sc}") for sc in range(NSC)]
                for ttile in range(ND):
                    for sc in range(NSC):
                        pg = psg_pool.tile([PT, SC], f32, tag="pg", name=f"pg{b}_{ttile}_{sc}")
                        for dk in range(ND):
                            nc.tensor.matmul(
                                pg[:],
                                xt_sb[dk][:, ttile * PT:(ttile + 1) * PT],
                                y_sb[dk][:, sc * SC:(sc + 1) * SC],
                                start=(dk == 0),
                                stop=(dk == ND - 1) and not with_a,
                            )
                        if with_a:
                            nc.tensor.matmul(
                                pg[:], ones_sb[:],
                                arow[:, sc * SC:(sc + 1) * SC],
                                start=False, stop=True,
                            )
                        tT = t_pool.tile([PT, SC], f32r, tag="tT", name=f"tT{b}_{ttile}_{sc}")
                        nc.scalar.activation(tT[:], pg[:], AF.Tanh)
                        vcol = vr_sb[:, ttile:ttile + 1].bitcast(f32)
                        if ttile == 0:
                            nc.vector.tensor_scalar_mul(
                                acc[sc][:], tT[:].bitcast(f32), vcol)
                        elif ttile < ND - 1:
                            nc.vector.scalar_tensor_tensor(
                                acc[sc][:], tT[:].bitcast(f32), vcol,
                                acc[sc][:], op0=ALU.mult, op1=ALU.add)
                        else:
                            # final accumulate writes the f32r matmul operand
                            nc.vector.scalar_tensor_tensor(
                                accr[sc][:], tT[:].bitcast(f32), vcol,
                                acc[sc][:], op0=ALU.mult, op1=ALU.add)
                # cross-partition sum of acc via ones-column matmul
                sv = psv_pool.tile([1, S], f32, tag="sv", name=f"sv{b}")
                for sc in range(NSC):
                    nc.tensor.matmul(sv[:, sc * SC:(sc + 1) * SC],
                                     onescol[:], accr[sc][:],
                                     start=True, stop=True)

                # ---- Phase C: softmax over sv row; out = sum_s w[s] X[:, s]
                negm = small_pool.tile([1, 1], f32, tag="negm", name=f"negm{b}")
                nc.vector.reduce_max(negm[:], sv[:], axis=AX.X, negate=True)
                erow = row_pool.tile([1, S], f32r, tag="erow", name=f"erow{b}")
                zsum = small_pool.tile([1, 1], f32, tag="zsum", name=f"zsum{b}")
                nc.scalar.activation(erow[:], sv[:], AF.Exp,
                                     bias=negm[:], accum_out=zsum[:])
                # normalization (the 1/Z divide) happens on the host:
                # broadcast unnormalized exp weights, emit Z separately
                nc.sync.dma_start(zn_d.ap()[b:b + 1, :], zsum[:])
                pw = psw_pool.tile([PT, S], f32, tag="pw", name=f"pw{b}")
                for sc in range(NSC):
                    nc.tensor.matmul(
                        pw[:, sc * SC:(sc + 1) * SC],
                        ones_sb[:],
                        erow[:, sc * SC:(sc + 1) * SC],
                        start=True, stop=True,
                    )
                # fused multiply+free-dim-sum on the DVE, reading pw PSUM
                oc = oc_pool.tile([PT, ND], f32, tag="oc", name=f"oc{b}")
                for dk in range(ND):
                    scr = scr_pool.tile([PT, S], f32, tag="scr", name=f"scr{b}_{dk}")
                    nc.vector.scalar_tensor_tensor(
                        scr[:], xt_sb[dk][:].bitcast(f32), 1.0, pw[:],
                        op0=ALU.mult, op1=ALU.mult,
                        accum_out=oc[:, dk:dk + 1])
                nc.sync.dma_start(
                    out_d.ap()[b].rearrange("(i p) -> p i", p=PT), oc[:])

    nc.compile()
    return nc


_CACHE: dict = {}


def _get_nc(with_u2: bool, with_a: bool, c_bias: float):
    key = (with_u2, with_a, c_bias if with_a else 0.0)
    if key not in _CACHE:
        _CACHE[key] = _build(with_u2, with_a, c_bias)
    return _CACHE[key]


def kernel(x, Wq, bq, Wk, bk, v):
    x = np.asarray(x, dtype=np.float32)
    Wq = np.asarray(Wq, dtype=np.float32)
    bq = np.asarray(bq, dtype=np.float32)
    Wk = np.asarray(Wk, dtype=np.float32)
    bk = np.asarray(bk, dtype=np.float32)
    v = np.asarray(v, dtype=np.float32)

    # host-side algebra (small, fp64 for accuracy)
    M = (Wq.astype(np.float64).T @ Wk.astype(np.float64)).astype(np.float32)
    u2 = (Wk.astype(np.float64).T @ bq.astype(np.float64)).astype(np.float32)
    u1 = (Wq.astype(np.float64).T @ bk.astype(np.float64)).astype(np.float32)
    c = float(bq.astype(np.float64) @ bk.astype(np.float64))

    with_u2 = bool(np.any(u2))
    with_a = bool(np.any(u1)) or c != 0.0

    # [dp, dk, 128, 128] blocks: blk[dp, dk] = M[dk*128:.., dp*128:..]
    m_blocks = np.ascontiguousarray(
        M.reshape(ND, PT, ND, PT).transpose(2, 0, 1, 3))
    m_r = _rne12(m_blocks)
    vr = _rne12(np.ascontiguousarray(v.reshape(ND, PT).T))
    u2r = np.ascontiguousarray(u2.reshape(ND, PT).T)
    u1r = _rne12(np.ascontiguousarray(u1.reshape(ND, PT).T))

    nc = _get_nc(with_u2, with_a, c)

    in_maps = []
    for core in range(NCORES):
        xs = x[core * BL:(core + 1) * BL]              # [BL, S, D]
        xts = _rne12(np.ascontiguousarray(xs.transpose(0, 2, 1)))  # [BL, D, S]
        im = {"xt": xts, "m": m_r, "vr": vr}
        if with_u2:
            im["u2r"] = u2r
        if with_a:
            im["u1r"] = u1r
        in_maps.append(im)

    global _LAST_IN_MAPS
    _LAST_IN_MAPS = in_maps
    last_exc = None
    for attempt in range(3):
        try:
            res = run_bass_kernel_spmd(nc, in_maps,
                                       core_ids=list(range(NCORES)),
                                       trace=False)
            break
        except Exception as e:  # transient device errors: back off and retry
            last_exc = e
            import time as _time
            _time.sleep(5 * (attempt + 1))
    else:
        raise last_exc
    out = np.concatenate([res.results[i]["out"] for i in range(NCORES)], axis=0)
    zn = np.concatenate([res.results[i]["zn"] for i in range(NCORES)], axis=0)
    out = out / zn
    return out.astype(np.float32)
